# revision 1
# baseline (speedup 1.0000x reference)
"""Trainium2 Bass kernel for GNN message passing (8 NeuronCores, SPMD).

    out = segment_sum(x[src] @ W, tgt, N) + x @ W_self

Key algebraic identity: segment_sum(x[src] @ W, tgt) = segment_sum(x[src], tgt) @ W,
so the per-edge matmul hoists out of the reduction (21 GFLOP -> 6.6 GFLOP).

Sharding: target nodes are split into 8 contiguous ranges of 12500 (one per
core); edges are bucketed to the core owning their target. x is replicated in
every core's HBM so each core gathers arbitrary source rows locally (no
cross-core halo exchange needed under the full-I/O contract).

Per core, working transposed throughout (out.T = W.T @ hT + W_self.T @ xT):
  - targets are processed in 98 windows of 128 nodes
  - per 128-edge tile: G[e,f] = x[src_e] via indirect DMA gather,
    S[e,n] = onehot(tgt_local) built by a DVE is_equal against an iota,
    hT (PSUM) += matmul(lhsT=G, rhs=S)   # = sum_e G[e,f] S[e,n]
  - outT (PSUM) = matmul(lhsT=W, rhs=hT) + matmul(lhsT=W_self, rhs=xT_window)
The host transposes per-core [128, 12544] outputs back and concatenates.
"""

import numpy as np

P = 128
D = 128
N_NODES = 100000
N_CORES = 8
N_LOC = N_NODES // N_CORES          # 12500
N_WIN = (N_LOC + P - 1) // P        # 98
N_PAD = N_WIN * P                   # 12544

# dma_gather uses int16 row indices, so x is addressed through 4 overlapping
# 32768-row chunks; every source row is reachable from >=1 chunk and rows in
# overlap regions can go to either side, which lets the host balance the four
# per-window runs under the per-chunk tile cap.
N_CHUNK = 4
CHUNK_SPAN = 32768
CHUNK_BASE = [0, 22411, 44822, N_NODES - CHUNK_SPAN]

_program_cache: dict = {}


def _build_program(
    t_win: int,
    reps: int = 1,
    n_queues: int = 2,
    act_copy: bool = True,
    w_group: int = 1,
    split16: bool = False,
):
    import concourse.bass as bass
    import concourse.mybir as mybir
    import concourse.tile as tile
    from concourse.bacc import Bacc

    f32 = mybir.dt.float32
    t_tot = N_WIN * t_win

    # consts packed as one tensor/DMA so consumers wait on a single semaphore:
    # [idx16 (int16 bits) | tl | iota | W | W_self] along the free dim
    t_c = t_win // N_CHUNK
    idx_cols16 = N_WIN * N_CHUNK * t_c * 8   # int16 columns
    idx_cols = idx_cols16 // 2               # as float32 columns
    k_const = idx_cols + t_tot + 3 * P

    # Bacc (not raw Bass): its finalize() legalizes sync waits — TRN2 allows
    # at most one semaphore wait per instruction and walrus rejects more.
    nc = Bacc(num_swdge_queues=n_queues)
    bf16 = mybir.dt.bfloat16
    if split16:
        # x pre-split on host into [bf16(x) | bf16(x - bf16(x))] per row: the
        # aggregation runs as two bf16 matmuls (G_hi + G_lo vs exact-bf16 S),
        # streaming at 1 cy/row each vs fp32's 4 cy/row — 2x PE with ~17-bit
        # effective mantissa. Same gather descriptor count and bytes (512B/row).
        xs_d = nc.declare_dram_parameter(
            "xs", [N_NODES, 2 * D], bf16, isOutput=False
        )
    else:
        x_d = nc.declare_dram_parameter("x", [N_NODES, D], f32, isOutput=False)
    xT_d = nc.declare_dram_parameter("xT", [D, N_PAD], f32, isOutput=False)
    consts_d = nc.declare_dram_parameter(
        "consts", [P, k_const], mybir.dt.int32, isOutput=False
    )
    outT_d = nc.declare_dram_parameter("outT", [D, N_PAD], f32, isOutput=True)

    with tile.TileContext(nc) as tc:
        with (
            tc.tile_pool(name="const", bufs=1) as cpool,
            tc.tile_pool(name="gath", bufs=16) as gpool,
            tc.tile_pool(name="spool", bufs=3) as spool,
            tc.tile_pool(name="wtile", bufs=3) as wpool,
            tc.tile_pool(name="psum", bufs=2, space="PSUM") as psum,
            tc.tile_pool(name="opsum", bufs=2, space="PSUM") as opsum,
            tc.tile_pool(name="scratch", bufs=1, space="PSUM") as scratch_pool,
        ):
            scratch_ps = scratch_pool.tile([1, 1], f32)
            const_sb = cpool.tile([P, k_const], mybir.dt.int32)
            nc.sync.dma_start(const_sb[:], consts_d[:])
            idx16_sb = const_sb[:, 0:idx_cols].bitcast(mybir.dt.int16)
            tl_sb = const_sb[:, idx_cols : idx_cols + t_tot].bitcast(f32)
            iota_sb = const_sb[:, idx_cols + t_tot : idx_cols + t_tot + P].bitcast(f32)
            w_sb = const_sb[:, idx_cols + t_tot + P : idx_cols + t_tot + 2 * P].bitcast(
                f32
            )
            ws_sb = const_sb[
                :, idx_cols + t_tot + 2 * P : idx_cols + t_tot + 3 * P
            ].bitcast(f32)

            s_dt = bf16 if split16 else f32
            for w in [w for _ in range(reps) for w in range(N_WIN)]:
                hT_ps = psum.tile([D, P], f32)
                S_big = spool.tile([P, t_win, P], s_dt)
                nc.vector.tensor_tensor(
                    out=S_big[:],
                    in0=tl_sb[:, w * t_win : (w + 1) * t_win, None].to_broadcast(
                        [P, t_win, P]
                    ),
                    in1=iota_sb[:, None, :].to_broadcast([P, t_win, P]),
                    op=mybir.AluOpType.is_equal,
                )
                # fp32 matmuls are single fused instructions that can carry only
                # ONE sync wait; the first real matmul below depends on both
                # S_big (DVE) and G_big (DMA). This throwaway 1x1 matmul makes
                # the PE queue observe the DVE tick first so each real matmul
                # needs a single wait.
                nc.tensor.matmul(
                    scratch_ps[:],
                    lhsT=S_big[:, 0, 0:1],
                    rhs=S_big[:, 0, 0:1],
                    start=True,
                    stop=True,
                )
                # gather via dma_gather (int16 idx against a 32768-row chunk of
                # x): G_big slot (p, t) = row idx[t*128+p] of the chunk. Much
                # cheaper on the GPSIMD Q7 descriptor generator than per-tile
                # indirect_dma_start. (A single batched indirect DMA with a
                # [128, t_win] offset AP computes garbage on real HW.)
                gd = 2 * D if split16 else D
                G_big = gpool.tile([P, t_win, gd], s_dt)
                for c in range(N_CHUNK):
                    g = w * N_CHUNK + c
                    nc.gpsimd.dma_gather(
                        G_big[:, c * t_c : (c + 1) * t_c, :],
                        (xs_d if split16 else x_d)[
                            CHUNK_BASE[c] : CHUNK_BASE[c] + CHUNK_SPAN, :
                        ],
                        idx16_sb[:, g * t_c * 8 : (g + 1) * t_c * 8],
                        t_c * P,
                        t_c * P,
                        gd,
                        queue_num=c % n_queues,
                    )
                for t in range(t_win):
                    if split16:
                        nc.tensor.matmul(
                            hT_ps[:],
                            lhsT=G_big[:, t, 0:D],
                            rhs=S_big[:, t, :],
                            start=(t == 0),
                            stop=False,
                        )
                        nc.tensor.matmul(
                            hT_ps[:],
                            lhsT=G_big[:, t, D : 2 * D],
                            rhs=S_big[:, t, :],
                            start=False,
                            stop=(t == t_win - 1),
                        )
                    else:
                        nc.tensor.matmul(
                            hT_ps[:],
                            lhsT=G_big[:, t, :],
                            rhs=S_big[:, t, :],
                            start=(t == 0),
                            stop=(t == t_win - 1),
                        )
                if w_group == 1:
                    hT_sb = wpool.tile([D, P], f32, tag="hT")
                    nc.vector.tensor_copy(hT_sb[:], hT_ps[:])
                    xT_sb = wpool.tile([D, P], f32, tag="xT")
                    nc.sync.dma_start(xT_sb[:], xT_d[:, w * P : (w + 1) * P])
                    outT_ps = opsum.tile([D, P], f32)
                    nc.tensor.matmul(
                        outT_ps[:], lhsT=w_sb, rhs=hT_sb[:], start=True, stop=False
                    )
                    nc.tensor.matmul(
                        outT_ps[:], lhsT=ws_sb, rhs=xT_sb[:], start=False, stop=True
                    )
                    o_sb = wpool.tile([D, P], f32, tag="o")
                    if act_copy:
                        # ACT is otherwise idle; taking the outT copy off DVE
                        # (which builds every S one-hot) relieves the
                        # 2nd-busiest engine despite slower per-op copies.
                        nc.scalar.copy(o_sb[:], outT_ps[:])
                    else:
                        nc.vector.tensor_copy(o_sb[:], outT_ps[:])
                    nc.sync.dma_start(outT_d[:, w * P : (w + 1) * P], o_sb[:])
                    continue
                # grouped W-apply: stage hT of w_group windows side by side,
                # then stream both weight matmuls at N = w_group*128 to
                # amortize the fp32 weight loads (no FWL for fp32)
                gi = w % w_group
                if gi == 0:
                    n_in_grp = min(w_group, N_WIN - w)
                    hT_sb = wpool.tile([D, w_group * P], f32, tag="hT")
                nc.vector.tensor_copy(
                    hT_sb[:, gi * P : (gi + 1) * P], hT_ps[:]
                )
                if gi == n_in_grp - 1:
                    w0 = w - gi
                    span = n_in_grp * P
                    xT_sb = wpool.tile([D, w_group * P], f32, tag="xT")
                    nc.sync.dma_start(
                        xT_sb[:, :span], xT_d[:, w0 * P : w0 * P + span]
                    )
                    outT_ps = opsum.tile([D, w_group * P], f32)
                    nc.tensor.matmul(
                        outT_ps[:, :span],
                        lhsT=w_sb,
                        rhs=hT_sb[:, :span],
                        start=True,
                        stop=False,
                    )
                    nc.tensor.matmul(
                        outT_ps[:, :span],
                        lhsT=ws_sb,
                        rhs=xT_sb[:, :span],
                        start=False,
                        stop=True,
                    )
                    o_sb = wpool.tile([D, w_group * P], f32, tag="o")
                    if act_copy:
                        nc.scalar.copy(o_sb[:, :span], outT_ps[:, :span])
                    else:
                        nc.vector.tensor_copy(o_sb[:, :span], outT_ps[:, :span])
                    nc.sync.dma_start(
                        outT_d[:, w0 * P : w0 * P + span], o_sb[:, :span]
                    )

    nc.finalize()
    return nc


def _prep_inputs(x, edge_index, W, W_self):
    """Host-side sharding: bucket+sort edges by target core/window, pad to a
    uniform tile count, build per-core input maps."""
    x = np.ascontiguousarray(np.asarray(x, dtype=np.float32))
    W = np.ascontiguousarray(np.asarray(W, dtype=np.float32))
    W_self = np.ascontiguousarray(np.asarray(W_self, dtype=np.float32))
    ei = np.asarray(edge_index)
    src = ei[0].astype(np.int64)
    tgt = ei[1].astype(np.int64)
    E = src.shape[0]

    order = np.argsort(tgt, kind="stable")
    src_s = src[order].astype(np.int64)
    tgt_s = tgt[order]
    core = tgt_s // N_LOC
    wloc = (tgt_s - core * N_LOC) // P
    gw = (core * N_WIN + wloc).astype(np.int64)
    counts = np.bincount(gw, minlength=N_CORES * N_WIN)
    t_win_data = max(1, int(np.ceil(counts.max() / P)))
    t_c = max(2, (t_win_data + N_CHUNK - 1) // N_CHUNK)

    # chunk feasibility per edge: lo = highest chunk with base <= s,
    # hi = lowest chunk with s < base + CHUNK_SPAN (consecutive range)
    bases = np.asarray(CHUNK_BASE, np.int64)
    lo = np.searchsorted(bases, src_s, side="right") - 1
    hi = np.searchsorted(bases + CHUNK_SPAN, src_s, side="right")
    starts = np.concatenate([[0], np.cumsum(counts)])
    tl_val = (tgt_s - (core * N_LOC + wloc * P)).astype(np.float32)

    while True:
        cap = t_c * P
        t_win = N_CHUNK * t_c
        t_tot = N_WIN * t_win
        idx16 = np.zeros((N_CORES, N_WIN * N_CHUNK * cap // 16, 16), np.int16)
        tl_flat = np.full(N_CORES * t_tot * P, -1.0, np.float32)
        ok = True
        for g in range(N_CORES * N_WIN):
            a, b = starts[g], starts[g + 1]
            if b - a > N_CHUNK * cap:
                ok = False
                break
            s_g, hi_g, lo_g, tl_g = src_s[a:b], hi[a:b], lo[a:b], tl_val[a:b]
            taken = np.zeros(b - a, bool)
            c_core, w = divmod(g, N_WIN)
            for c in range(N_CHUNK):
                cand = (~taken) & (hi_g <= c) & (c <= lo_g)
                must = cand & (lo_g == c)
                n_must = int(must.sum())
                if n_must > cap:
                    ok = False
                    break
                sel = must.nonzero()[0]
                flex = (cand & ~must).nonzero()[0][: cap - n_must]
                pick = np.concatenate([sel, flex])
                taken[pick] = True
                n = pick.size
                idx = (s_g[pick] - bases[c]).astype(np.int16)
                # wrapped int16 layout: slot s -> [s % 16, s // 16]
                blk = np.zeros(cap, np.int16)
                blk[:n] = idx
                row0 = (w * N_CHUNK + c) * (cap // 16)
                idx16[c_core, row0 : row0 + cap // 16] = blk.reshape(cap // 16, 16)
                # tl slots for this chunk run (pads stay -1)
                base_slot = g * (t_win * P) + c * cap
                tl_flat[base_slot : base_slot + n] = tl_g[pick]
            if not ok or not taken.all():
                ok = ok and bool(taken.all())
                if not ok:
                    break
        if ok:
            break
        t_c += 1

    tl_dev = tl_flat.reshape(N_CORES, t_tot, P).transpose(0, 2, 1)
    iota = np.tile(np.arange(P, dtype=np.float32), (P, 1))
    in_maps = []
    for c in range(N_CORES):
        # idx16[c]: [n16, 16] with slot s of block g at [g*cap/16 + s%16 ...]
        # -> SBUF layout [128 partitions, cols]: block g occupies columns
        # [g*t_c*8, (g+1)*t_c*8), partition rows 0..15
        n_blocks = N_WIN * N_CHUNK
        cols16 = t_c * 8
        a = idx16[c].reshape(n_blocks, cap // 16, 16)  # [g, col, row]
        # [16, cols] block replicated across all 8 GPSIMD Q7 cores' stripes
        sb = np.tile(a.transpose(2, 0, 1).reshape(16, n_blocks * cols16), (8, 1))
        if c == 0:
            import ml_dtypes

            x_hi = x.astype(ml_dtypes.bfloat16)
            x_lo = (x - x_hi.astype(np.float32)).astype(ml_dtypes.bfloat16)
            xs = np.concatenate([x_hi, x_lo], axis=1)
        xT_c = np.zeros((D, N_PAD), np.float32)
        xT_c[:, :N_LOC] = x[c * N_LOC : (c + 1) * N_LOC].T
        consts = np.concatenate(
            [
                sb.view(np.int32),
                tl_dev[c].view(np.int32),
                iota.view(np.int32),
                W.view(np.int32),
                W_self.view(np.int32),
            ],
            axis=1,
        )
        in_maps.append({"x": x, "xs": xs, "xT": xT_c, "consts": consts})
    return in_maps, t_win


def run(x, edge_index, W, W_self, trace=False, **trace_kwargs):
    """Returns (output [100000,128] float32, BassKernelResults)."""
    from concourse import bass_utils

    in_maps, t_win = _prep_inputs(x, edge_index, W, W_self)
    nc = _program_cache.get(t_win)
    if nc is None:
        nc = _build_program(t_win)
        _program_cache[t_win] = nc
    # A NeuronCore occasionally comes up wedged from a previous session
    # (NRT_EXEC_UNIT_UNRECOVERABLE); the failed attempt itself clears it, so
    # one retry recovers.
    try:
        res = bass_utils.run_bass_kernel_spmd(
            nc, in_maps, core_ids=list(range(N_CORES)), trace=trace, **trace_kwargs
        )
    except Exception:
        res = bass_utils.run_bass_kernel_spmd(
            nc, in_maps, core_ids=list(range(N_CORES)), trace=trace, **trace_kwargs
        )
    out = np.empty((N_NODES, D), np.float32)
    for c in range(N_CORES):
        out[c * N_LOC : (c + 1) * N_LOC] = res.results[c]["outT"].T[:N_LOC]
    return out, res


def kernel(x, edge_index, W, W_self):
    out, _ = run(x, edge_index, W, W_self, trace=False)
    return out



# revision 2
# speedup vs baseline: 1.1068x; 1.1068x over previous
"""Trainium2 Bass kernel for GNN message passing (8 NeuronCores, SPMD) — v3.

    out = segment_sum(x[src] @ W, tgt, N) + x @ W_self

Identity: segment_sum(x[src] @ W, tgt) = segment_sum(x[src], tgt) @ W, so the
per-edge matmul hoists out of the reduction.

Cost structure on this silicon (measured via pipelined R-slope microbench):
a dma_gather costs ~1 us fixed + ~4.3 ns/idx of Q7 descriptor-generation,
executed by the single core pair (2q, 2q+1) owning its queue; instructions on
different queues overlap ~2.6x when the WAR horizon (G pool depth) allows.
The SDMA drain itself is cheap. So the gather plan is:
  - one call per (3-window group, chunk): 768 idxs/call, under the ~1008
    descriptor-ring cap (64 descs/engine) that wedges bigger calls;
  - queue = chunk index with num_swdge_queues=4, so the 4 calls of a group
    land on all four Q7 core pairs concurrently;
  - gpool bufs=6 keeps the WAR dependency ~6 groups ahead so the POOL
    sequencer never blocks on gather consumers;
  - pads gather row 0 of their chunk (safe, DRAM-page-hit cheap). The
    ucode's trailing-negative skip is NOT used: it desyncs the persistent
    DGE ring bookkeeping across executions (decode pushes pre-skip counts,
    the Q7 writes post-skip) and wedges the second run.

Everything else: bf16 end-to-end (gather 256 B/row, one-hot S via DVE
2x_1P is_equal with materialized iota, matmuls bf16 with fp32 PSUM, bf16
output converted on host), hT/outT copies on ACT, xT preloaded, W-apply
grouped at N=512.
"""

import numpy as np

P = 128
D = 128
N_NODES = 100000
N_CORES = 8
N_LOC = N_NODES // N_CORES          # 12500
N_WIN = (N_LOC + P - 1) // P        # 98
N_PAD = N_WIN * P                   # 12544

# dma_gather uses int16 row indices, so x is addressed through 4 overlapping
# 32768-row chunks; every source row is reachable from >=1 chunk and rows in
# overlap regions can go to either side, which lets the host balance the four
# per-window runs under the per-chunk tile cap.
N_CHUNK = 4
CHUNK_SPAN = 32768
CHUNK_BASE = [0, 22411, 44822, N_NODES - CHUNK_SPAN]

G_WIN = 3                           # windows per gather group (98 = 32*3 + 2)

_program_cache: dict = {}


def _group_sizes(g_win):
    sizes = [g_win] * (N_WIN // g_win)
    if N_WIN % g_win:
        sizes.append(N_WIN % g_win)
    return sizes


def _build_program(
    t_win: int,
    reps: int = 1,
    n_queues: int = 4,
    w_group: int = 4,
    g_win: int = G_WIN,
    g_bufs: int = 6,
    split_idx: bool = True,
):
    import concourse.mybir as mybir
    import concourse.tile as tile
    from concourse.bacc import Bacc

    f32 = mybir.dt.float32
    bf16 = mybir.dt.bfloat16
    t_c = t_win // N_CHUNK
    sizes = _group_sizes(g_win)

    # consts packed as [idx16 | tl (bf16) | iota_nt (bf16) | W | Ws]
    idx_cols16 = N_WIN * N_CHUNK * t_c * 8          # int16 columns
    idx_cols = idx_cols16 // 2                      # as int32 columns
    t_tot = N_WIN * t_win
    tl_cols = t_tot // 2                            # bf16 -> int32 columns
    iota_cols = t_win * P // 2
    w_cols = D // 2
    k_const = idx_cols + tl_cols + iota_cols + 2 * w_cols

    # Bacc (not raw Bass): its finalize() legalizes sync waits — TRN2 allows
    # at most one semaphore wait per instruction and walrus rejects more.
    nc = Bacc(num_swdge_queues=n_queues)
    xb_d = nc.declare_dram_parameter("xb", [N_NODES, D], bf16, isOutput=False)
    xT_d = nc.declare_dram_parameter("xT", [D, N_PAD], bf16, isOutput=False)
    consts_d = nc.declare_dram_parameter(
        "consts", [P, k_const], mybir.dt.int32, isOutput=False
    )
    # bf16 output halves the store stream; host converts back to fp32
    # (adds ~2^-9 relative rounding, well inside the 2e-2 gate)
    outT_d = nc.declare_dram_parameter("outT", [D, N_PAD], bf16, isOutput=True)

    with tile.TileContext(nc) as tc:
        with (
            tc.tile_pool(name="const", bufs=1) as cpool,
            tc.tile_pool(name="gath", bufs=g_bufs) as gpool,
            tc.tile_pool(name="spool", bufs=4) as spool,
            tc.tile_pool(name="wtile", bufs=3) as wpool,
            tc.tile_pool(name="psum", bufs=2, space="PSUM") as psum,
            tc.tile_pool(name="opsum", bufs=2, space="PSUM") as opsum,
            tc.tile_pool(name="scratch", bufs=1, space="PSUM") as scratch_pool,
        ):
            scratch_ps = scratch_pool.tile([1, 1], f32)
            if split_idx:
                # idx16 in its own tile + DMA: gathers depend only on it, so
                # they start before the (larger) tl/iota/W/xT streams land.
                idx_t = cpool.tile([P, idx_cols], mybir.dt.int32)
                nc.sync.dma_start(idx_t[:], consts_d[:, :idx_cols])
                rest_t = cpool.tile([P, k_const - idx_cols], mybir.dt.int32)
                nc.sync.dma_start(rest_t[:], consts_d[:, idx_cols:])
                idx16_sb = idx_t[:].bitcast(mybir.dt.int16)
                o = 0
            else:
                rest_t = cpool.tile([P, k_const], mybir.dt.int32)
                nc.sync.dma_start(rest_t[:], consts_d[:])
                idx16_sb = rest_t[:, :idx_cols].bitcast(mybir.dt.int16)
                o = idx_cols
            xT_sb = cpool.tile([D, N_PAD], bf16)
            nc.sync.dma_start(xT_sb[:], xT_d[:])

            tl_sb = rest_t[:, o : o + tl_cols].bitcast(bf16)
            o += tl_cols
            iota_sb = rest_t[:, o : o + iota_cols].bitcast(bf16)
            o += iota_cols
            w_sb = rest_t[:, o : o + w_cols].bitcast(bf16)
            o += w_cols
            ws_sb = rest_t[:, o : o + w_cols].bitcast(bf16)
            # iota_nt[p, n, t] = n  (constant along t) — materialized so the
            # is_equal's in0 has inner step 1 (2x_1P eligibility).
            iota_nt = iota_sb.rearrange("p (n t) -> p n t", t=t_win)

            for rep in range(reps):
                w0 = 0
                for grp, g_sz in enumerate(sizes):
                    # one gather call per chunk covers all g_sz windows:
                    # G_big[:, c, wl*t_c + i, :] = 128 gathered rows (bf16)
                    # for window w0+wl, chunk c, tile i.
                    G_big = gpool.tile([P, N_CHUNK, g_win * t_c, D], bf16, tag="G")
                    for c in range(N_CHUNK):
                        nidx = g_sz * t_c * P
                        cw0 = (w0 * N_CHUNK + g_sz * c) * (t_c * 8)
                        nc.gpsimd.dma_gather(
                            G_big[:, c, 0 : g_sz * t_c, :],
                            xb_d[CHUNK_BASE[c] : CHUNK_BASE[c] + CHUNK_SPAN, :],
                            idx16_sb[:, cw0 : cw0 + g_sz * t_c * 8],
                            nidx,
                            nidx,
                            D,
                            queue_num=c % n_queues,
                        )
                    for wl in range(g_sz):
                        w = w0 + wl
                        hT_ps = psum.tile([D, P], f32)
                        # S[p, n, t] one-hot: tl broadcast along n (outer,
                        # step 0), inner t step 1 -> DVE 2x_1P.
                        S_big = spool.tile([P, P, t_win], bf16)
                        nc.vector.tensor_tensor(
                            out=S_big[:],
                            in0=iota_nt,
                            in1=tl_sb[
                                :, None, w * t_win : (w + 1) * t_win
                            ].to_broadcast([P, P, t_win]),
                            op=mybir.AluOpType.is_equal,
                        )
                        # bf16 matmuls carry only ONE sync wait; the first real
                        # matmul below depends on both S_big (DVE) and G_big
                        # (DMA). This throwaway 1x1 matmul makes the PE queue
                        # observe the DVE tick first so each real matmul needs
                        # a single wait.
                        nc.tensor.matmul(
                            scratch_ps[:],
                            lhsT=S_big[:, 0, 0:1],
                            rhs=S_big[:, 0, 0:1],
                            start=True,
                            stop=True,
                        )
                        for t in range(t_win):
                            c, i = divmod(t, t_c)
                            nc.tensor.matmul(
                                hT_ps[:],
                                lhsT=G_big[:, c, wl * t_c + i, :],
                                rhs=S_big[:, :, t],
                                start=(t == 0),
                                stop=(t == t_win - 1),
                            )
                        # grouped W-apply: stage hT of w_group windows side by
                        # side (bf16), then stream both weight matmuls at
                        # N = w_group*128.
                        gi = w % w_group
                        if gi == 0:
                            n_in_grp = min(w_group, N_WIN - w)
                            hT_sb = wpool.tile([D, w_group * P], bf16, tag="hT")
                        nc.scalar.copy(hT_sb[:, gi * P : (gi + 1) * P], hT_ps[:])
                        if gi == n_in_grp - 1:
                            wg0 = w - gi
                            span = n_in_grp * P
                            outT_ps = opsum.tile([D, w_group * P], f32)
                            nc.tensor.matmul(
                                outT_ps[:, :span],
                                lhsT=w_sb,
                                rhs=hT_sb[:, :span],
                                start=True,
                                stop=False,
                            )
                            nc.tensor.matmul(
                                outT_ps[:, :span],
                                lhsT=ws_sb,
                                rhs=xT_sb[:, wg0 * P : wg0 * P + span],
                                start=False,
                                stop=True,
                            )
                            o_sb = wpool.tile([D, w_group * P], bf16, tag="o")
                            nc.scalar.copy(o_sb[:, :span], outT_ps[:, :span])
                            nc.sync.dma_start(
                                outT_d[:, wg0 * P : wg0 * P + span],
                                o_sb[:, :span],
                            )
                    w0 += g_sz

    nc.finalize()
    return nc


def _prep_inputs(x, edge_index, W, W_self, g_win=G_WIN, neg_pads=False):
    """Host-side sharding: bucket+sort edges by target core/window, pad to a
    uniform tile count, build per-core input maps."""
    import ml_dtypes

    x = np.ascontiguousarray(np.asarray(x, dtype=np.float32))
    W = np.ascontiguousarray(np.asarray(W, dtype=np.float32))
    W_self = np.ascontiguousarray(np.asarray(W_self, dtype=np.float32))
    ei = np.asarray(edge_index)
    src = ei[0].astype(np.int64)
    tgt = ei[1].astype(np.int64)

    order = np.argsort(tgt, kind="stable")
    src_s = src[order].astype(np.int64)
    tgt_s = tgt[order]
    core = tgt_s // N_LOC
    wloc = (tgt_s - core * N_LOC) // P
    gw = (core * N_WIN + wloc).astype(np.int64)
    counts = np.bincount(gw, minlength=N_CORES * N_WIN)
    t_win_data = max(1, int(np.ceil(counts.max() / P)))
    t_c = max(2, (t_win_data + N_CHUNK - 1) // N_CHUNK)

    # chunk feasibility per edge: lo = highest chunk with base <= s,
    # hi = lowest chunk with s < base + CHUNK_SPAN (consecutive range)
    bases = np.asarray(CHUNK_BASE, np.int64)
    lo = np.searchsorted(bases, src_s, side="right") - 1
    hi = np.searchsorted(bases + CHUNK_SPAN, src_s, side="right")
    starts = np.concatenate([[0], np.cumsum(counts)])
    tl_val = (tgt_s - (core * N_LOC + wloc * P)).astype(np.float32)

    while True:
        cap = t_c * P
        t_win = N_CHUNK * t_c
        t_tot = N_WIN * t_win
        # idx_all[core, w, c, slot]; run_n[core, w, c] = real edge count
        idx_all = np.zeros((N_CORES, N_WIN, N_CHUNK, cap), np.int16)
        run_n = np.zeros((N_CORES, N_WIN, N_CHUNK), np.int32)
        tl_flat = np.full(N_CORES * t_tot * P, -1.0, np.float32)
        ok = True
        for g in range(N_CORES * N_WIN):
            a, b = starts[g], starts[g + 1]
            if b - a > N_CHUNK * cap:
                ok = False
                break
            s_g, hi_g, lo_g, tl_g = src_s[a:b], hi[a:b], lo[a:b], tl_val[a:b]
            taken = np.zeros(b - a, bool)
            c_core, w = divmod(g, N_WIN)
            for c in range(N_CHUNK):
                cand = (~taken) & (hi_g <= c) & (c <= lo_g)
                must = cand & (lo_g == c)
                n_must = int(must.sum())
                if n_must > cap:
                    ok = False
                    break
                sel = must.nonzero()[0]
                flex = (cand & ~must).nonzero()[0][: cap - n_must]
                pick = np.concatenate([sel, flex])
                taken[pick] = True
                n = pick.size
                idx_all[c_core, w, c, :n] = (s_g[pick] - bases[c]).astype(np.int16)
                run_n[c_core, w, c] = n
                # tl slots for this chunk run (pads stay -1)
                base_slot = g * (t_win * P) + c * cap
                tl_flat[base_slot : base_slot + n] = tl_g[pick]
            if not ok or not taken.all():
                ok = ok and bool(taken.all())
                if not ok:
                    break
        if ok:
            break
        t_c += 1

    sizes = _group_sizes(g_win)
    # tl as bf16 [P, t_tot] per core (slot (w, c, i, p) -> col w*t_win + c*t_c+i)
    tl_dev = np.ascontiguousarray(
        tl_flat.reshape(N_CORES, t_tot, P).transpose(0, 2, 1)
    ).astype(ml_dtypes.bfloat16)
    # iota_nt[p, n*t_win + t] = n (constant along t), identical per partition
    iota_nt = np.tile(
        np.repeat(np.arange(P, dtype=np.float32), t_win).astype(ml_dtypes.bfloat16),
        (P, 1),
    )
    x_b = x.astype(ml_dtypes.bfloat16)
    W_b = W.astype(ml_dtypes.bfloat16)
    Ws_b = W_self.astype(ml_dtypes.bfloat16)
    in_maps = []
    for cc in range(N_CORES):
        # per gather call (grp, c): slots of the group's windows' chunk-c runs
        # concatenated in window order. The call's trailing pads (last
        # window's run tail) are -1 so the ucode skips them; interior pads
        # stay 0 (gather row 0 — safe). Wrapped int16 layout within the
        # call: slot s -> [s % 16, s // 16], 16-partition block replicated
        # to all 8 Q7 core stripes.
        cols = []
        w0 = 0
        for g_sz in sizes:
            for c in range(N_CHUNK):
                blk = idx_all[cc, w0 : w0 + g_sz, c, :].copy()  # [g_sz, cap]
                if neg_pads:
                    n_last = run_n[cc, w0 + g_sz - 1, c]
                    blk[g_sz - 1, n_last:] = -1
                flat = blk.reshape(-1)
                cols.append(flat.reshape(-1, 16))
            w0 += g_sz
        wrapped = np.concatenate(cols, axis=0)        # [total/16, 16]
        sb = np.tile(wrapped.T.reshape(16, -1), (8, 1))
        xT_c = np.zeros((D, N_PAD), np.float32)
        xT_c[:, :N_LOC] = x[cc * N_LOC : (cc + 1) * N_LOC].T
        consts = np.concatenate(
            [
                sb.view(np.int32),
                tl_dev[cc].view(np.int32),
                iota_nt.view(np.int32),
                W_b.view(np.int32),
                Ws_b.view(np.int32),
            ],
            axis=1,
        )
        in_maps.append(
            {
                "xb": x_b,
                "xT": xT_c.astype(ml_dtypes.bfloat16),
                "consts": consts,
            }
        )
    return in_maps, t_win


def run(x, edge_index, W, W_self, trace=False, **trace_kwargs):
    """Returns (output [100000,128] float32, BassKernelResults)."""
    from concourse import bass_utils

    in_maps, t_win = _prep_inputs(x, edge_index, W, W_self)
    nc = _program_cache.get(t_win)
    if nc is None:
        nc = _build_program(t_win)
        _program_cache[t_win] = nc
    # A NeuronCore occasionally comes up wedged from a previous session
    # (NRT_EXEC_UNIT_UNRECOVERABLE); the failed attempt itself clears it, so
    # one retry recovers.
    try:
        res = bass_utils.run_bass_kernel_spmd(
            nc, in_maps, core_ids=list(range(N_CORES)), trace=trace, **trace_kwargs
        )
    except Exception:
        res = bass_utils.run_bass_kernel_spmd(
            nc, in_maps, core_ids=list(range(N_CORES)), trace=trace, **trace_kwargs
        )
    out = np.empty((N_NODES, D), np.float32)
    for c in range(N_CORES):
        out[c * N_LOC : (c + 1) * N_LOC] = (
            res.results[c]["outT"].astype(np.float32).T[:N_LOC]
        )
    return out, res


def kernel(x, edge_index, W, W_self):
    out, _ = run(x, edge_index, W, W_self, trace=False)
    return out
